# revision 45
# baseline (speedup 1.0000x reference)
"""GCNN message-passing layer on 8 Trainium2 NeuronCores (Bass/Tile).

Math (per token m):
    in_pot[m]  = (rep @ W_in)[head(m)] + b_in[lab(m)]
    in_gate[m] = (rep @ W_gate_in)[head(m)] + b_gate_in[lab(m)]
    self_pot   = rep @ W_self ; self_gate = rep @ W_gate_self
    w_d = sigmoid(gate_d) * msoft_d^2
    out = relu(in_pot*w_in + self_pot*w_self) * mask

Strategy: the gates are 2/514 of the FLOPs, so they're computed on the host
(one [M,512]@[512,2] BLAS call) along with the head gather and the mask
folding. The device input is a single K-stacked operand
    rep23[m] = [ w_in[m]*rep[head(m)] | w_self[m]*rep[m] ]  (K = 1024)
and the kernel reduces to relu(rep23 @ [W_in; W_self]) — one 8-chunk
PSUM-accumulating matmul chain plus one Relu per 128-token tile. No gather
matmuls, no sigmoid/copy tail: the PE array stays at its 2.4 GHz p-state
with nothing else on its critical path.

A nonzero b_in (general path) adds one more K chunk: rep23 gains
w_in[m]*onehot(lab(m)) rows and W2 gains the b_in rows.

Sharding: data-parallel over BNK (160 sentences / core), weights replicated.
The host gather is global, so arbitrary adjacency (even cross-sentence)
is supported.

rep23 ships as fp8-e3m4 (4 mantissa bits) scaled by 3 with the inverse
scale folded into the fp16 weights; W/out stay fp16. This halves the input
DMA (the bottleneck after the matmul restructure). REP_DT='f16' switches
back to all-fp16 if needed.
"""

import os

# Reset cores at device open: long-running sessions can leave the PE clock in
# a degraded state (~1.95 GHz instead of 2.4); a reset restores nominal.
os.environ.setdefault("NEURON_RT_RESET_CORES", "1")

import numpy as np
import ml_dtypes

import concourse.bass as bass  # noqa: F401  (kept for parity with bass_utils expectations)
import concourse.mybir as mybir
import concourse.tile as tile
from concourse import bacc, bass_utils

BNK, L, DIN, DOUT, NREL = 1280, 64, 512, 256, 40
NCORES = 8
SPC = BNK // NCORES          # sentences per core (160)
TOK = SPC * L                # tokens per core (10240)
TILE_T = 128                 # tokens per tile
NTILES = TOK // TILE_T       # 80
GROUP = 2                    # tiles per DMA batch (fine-grained: low head latency)
NG = NTILES // GROUP         # 40
KC = (2 * DIN) // 128        # K chunks for [rep2|rep3] (8)

F32 = mybir.dt.float32
F16 = mybir.dt.float16
F8E3 = mybir.dt.float8e3
AF = mybir.ActivationFunctionType

REP_DT = "f8e3"              # 'f8e3' (half DMA, rel err ~1.3e-2) or 'f16' (~5e-4)
F8_SCALE = 3.0               # rep23 pre-scale; inverse folded into fp16 W2
F8_MAX = 15.5                # e3m4 clamp

LAYOUT = "tm2"                # 'dm' (dout-major, 512-token matmuls) or 'tm' (token-major)
TB = 512                     # tokens per dout-major block (= one PSUM bank in f32)
NB = TOK // TB               # 20 blocks


def build_nc(rep_dt: str, with_bias: bool):
    """Per-core Bass program (identical on all cores)."""
    kc_tot = KC + 1 if with_bias else KC
    dt = F8E3 if rep_dt == "f8e3" else F16
    nc = bacc.Bacc("TRN2", target_bir_lowering=False, debug=False)

    repT_d = nc.dram_tensor("repT", [NG, 128, GROUP, kc_tot, TILE_T], dt, kind="ExternalInput")
    w2_d = nc.dram_tensor("w2", [128, kc_tot, DOUT], F16, kind="ExternalInput")
    out_d = nc.dram_tensor("out", [NG, 128, GROUP, DOUT], F16, kind="ExternalOutput")
    ka = kc_tot // 2  # first-arrival split point for tile 0 / w2

    with tile.TileContext(nc) as tc:
        with (
            tc.tile_pool(name="const", bufs=1) as const_pool,
            tc.tile_pool(name="rep", bufs=8) as rep_pool,
            tc.tile_pool(name="out", bufs=4) as out_pool,
            tc.tile_pool(name="psum", bufs=7, space="PSUM") as psum_pool,
            tc.tile_pool(name="dummy", bufs=1, space="PSUM") as dummy_psum_pool,
        ):
            # Split tile 0's rep and w2 into halves, issued interleaved, so the
            # first matmul chain starts as soon as the first half lands.
            # Dedicated const-pool tiles (not the rotating rep pool) so the
            # first-tile buffers can never alias the steady-state rotation.
            hs = [(0, ka), (ka, kc_tot - ka)]  # (start, len) per half
            rep0 = [const_pool.tile([128, 1, n, TILE_T], dt, name=f"rep0_{h}")
                    for h, (_, n) in enumerate(hs)]
            w2h = [const_pool.tile([128, n, DOUT], F16, name=f"w2_{h}")
                   for h, (_, n) in enumerate(hs)]
            rep0_v = repT_d[0]
            for h, (s, n) in enumerate(hs):
                # rep halves on the SP ring, w2 halves on the ACT ring so the
                # two DGE configs run concurrently at startup
                nc.sync.dma_start(rep0[h][:], rep0_v[:, 0:1, s:s + n, :])
                nc.scalar.dma_start(w2h[h][:], w2_d[:, s:s + n, :])

            # Warm-up matmuls on scratch SBUF (contents irrelevant — results
            # are never read): keep the PE continuously busy through the DMA
            # head so it reaches its full p-state clock before the real
            # chains begin.
            dummy_sb = const_pool.tile([128, DOUT], F16)
            nc.vector.memset(dummy_sb[:], 0.0)
            dummy_ps = dummy_psum_pool.tile([128, DOUT], F32)
            for _ in range(21):
                nc.tensor.matmul(dummy_ps[:], dummy_sb[:, 0:128], dummy_sb[:],
                                 start=True, stop=True)

            def w2_ap(kc):
                h = 0 if kc < ka else 1
                return w2h[h][:, kc - hs[h][0], :]

            for g in range(NG):
                if g == 0:
                    rep_sb = const_pool.tile([128, GROUP - 1, kc_tot, TILE_T], dt, name="rep0b")
                    nc.sync.dma_start(rep_sb[:], rep0_v[:, 1:GROUP])
                    tiles = [lambda kc: rep0[0 if kc < ka else 1][:, 0, kc - hs[0 if kc < ka else 1][0], :]] + [
                        (lambda ti_: lambda kc: rep_sb[:, ti_ - 1, kc, :])(t) for t in range(1, GROUP)]
                else:
                    rep_sb = rep_pool.tile([128, GROUP, kc_tot, TILE_T], dt)
                    nc.sync.dma_start(rep_sb[:], repT_d[g])
                    tiles = [(lambda ti_: lambda kc: rep_sb[:, ti_, kc, :])(t) for t in range(GROUP)]
                o_sb = out_pool.tile([128, GROUP, DOUT], F16)
                for ti in range(GROUP):
                    psum = psum_pool.tile([128, DOUT], F32)
                    for kc in range(kc_tot):
                        nc.tensor.matmul(psum[:], tiles[ti](kc), w2_ap(kc),
                                         start=kc == 0, stop=kc == kc_tot - 1)
                    nc.scalar.activation(o_sb[:, ti, :], psum[:], AF.Relu)
                    # last group drains per-tile on the ACT HWDGE ring for a
                    # shorter tail; earlier groups batch via gpsimd SWDGE below
                    if g == NG - 1:
                        nc.scalar.dma_start(out_d[g][:, ti, :], o_sb[:, ti, :])
                if g != NG - 1:
                    # output DMA via gpsimd SWDGE (idle engine); inputs ride
                    # the SP ring
                    nc.gpsimd.dma_start(out_d[g], o_sb[:])

    nc.compile()
    return nc


def build_nc_tm2(rep_dt: str, with_bias: bool):
    """Token-major with a kc-major phased start: tiles 0-3 ship chunk-pair-
    major and keep four PSUM chains open, so the first matmuls need only
    rep-phase0 (128 KB) + w2 quarter 0 (128 KB) and each 256 KB phase
    sustains 8 matmuls of consumption. Tiles 4+ run the proven tm loop."""
    kc_tot = KC + 1 if with_bias else KC
    assert kc_tot % 2 == 0, "phased start assumes even chunk count"
    dt = F8E3 if rep_dt == "f8e3" else F16
    nphase = kc_tot // 2
    nc = bacc.Bacc("TRN2", target_bir_lowering=False, debug=False)

    ng = (NTILES - 4) // GROUP  # groups for tiles 4+
    rep0x_d = nc.dram_tensor("rep0x", [nphase, 128, 2, 4, TILE_T], dt, kind="ExternalInput")
    repT_d = nc.dram_tensor("repT", [ng, 128, GROUP, kc_tot, TILE_T], dt, kind="ExternalInput")
    w2_d = nc.dram_tensor("w2", [128, kc_tot, DOUT], F16, kind="ExternalInput")
    out_d = nc.dram_tensor("out", [2 + ng, 128, GROUP, DOUT], F16, kind="ExternalOutput")

    with tile.TileContext(nc) as tc:
        with (
            tc.tile_pool(name="const", bufs=1) as const_pool,
            tc.tile_pool(name="rep", bufs=8) as rep_pool,
            tc.tile_pool(name="out", bufs=4) as out_pool,
            tc.tile_pool(name="psum", bufs=7, space="PSUM") as psum_pool,
            tc.tile_pool(name="dummy", bufs=1, space="PSUM") as dummy_psum_pool,
        ):
            # startup stream: rep pieces on the SP ring, w2 pieces on ACT.
            # Chunks 0 and 1 ship as single-chunk pieces (128 KB each with
            # their w2 slice) so the chains start as early as possible; the
            # rest in 2-chunk pieces.
            piece_sizes = [1, 1] + [2] * ((kc_tot - 2) // 2)
            rep0_ap_tbl, w2_ap_tbl = [None] * kc_tot, [None] * kc_tot
            s = 0
            for pi, n in enumerate(piece_sizes):
                rx = const_pool.tile([128, n, 4, TILE_T], dt, name=f"rep0x{pi}")
                wq = const_pool.tile([128, n, DOUT], F16, name=f"w2q{pi}")
                ph, kcc = s // 2, s % 2
                if n == 1:
                    src = rep0x_d[ph][:, kcc:kcc + 1]
                else:
                    assert kcc == 0 and n == 2
                    src = rep0x_d[ph]
                nc.sync.dma_start(rx[:], src)
                nc.scalar.dma_start(wq[:], w2_d[:, s:s + n, :])
                for off in range(n):
                    rep0_ap_tbl[s + off] = (rx, off)
                    w2_ap_tbl[s + off] = (wq, off)
                s += n

            dummy_sb = const_pool.tile([128, DOUT], F16)
            nc.vector.memset(dummy_sb[:], 0.0)
            dummy_ps = dummy_psum_pool.tile([128, DOUT], F32)
            for _ in range(11):
                nc.tensor.matmul(dummy_ps[:], dummy_sb[:, 0:128], dummy_sb[:],
                                 start=True, stop=True)

            # tiles 0-3: four open chains, chunk-major/tile-minor so
            # consecutive matmuls rotate across the four PSUM banks and each
            # arriving chunk unlocks four matmuls
            ps4 = [psum_pool.tile([128, DOUT], F32, name="psum") for _ in range(4)]
            o4 = [out_pool.tile([128, GROUP, DOUT], F16, name=f"o4_{gg}") for gg in range(2)]
            for kc in range(kc_tot):
                rx, roff = rep0_ap_tbl[kc]
                wq, woff = w2_ap_tbl[kc]
                for ti in range(4):
                    nc.tensor.matmul(ps4[ti][:], rx[:, roff, ti, :], wq[:, woff, :],
                                     start=kc == 0, stop=kc == kc_tot - 1)
            for ti in range(4):
                nc.scalar.activation(o4[ti // 2][:, ti % 2, :], ps4[ti][:], AF.Relu)
            for gg in range(2):
                nc.gpsimd.dma_start(out_d[gg], o4[gg][:])

            def w2_ap(kc):
                wq, woff = w2_ap_tbl[kc]
                return wq[:, woff, :]

            for g in range(ng):
                rep_sb = rep_pool.tile([128, GROUP, kc_tot, TILE_T], dt)
                nc.sync.dma_start(rep_sb[:], repT_d[g])
                o_sb = out_pool.tile([128, GROUP, DOUT], F16)
                for ti in range(GROUP):
                    psum = psum_pool.tile([128, DOUT], F32)
                    for kc in range(kc_tot):
                        nc.tensor.matmul(psum[:], rep_sb[:, ti, kc, :], w2_ap(kc),
                                         start=kc == 0, stop=kc == kc_tot - 1)
                    nc.scalar.activation(o_sb[:, ti, :], psum[:], AF.Relu)
                    if g == ng - 1:
                        nc.scalar.dma_start(out_d[2 + g][:, ti, :], o_sb[:, ti, :])
                if g != ng - 1:
                    nc.gpsimd.dma_start(out_d[2 + g], o_sb[:])

    nc.compile()
    return nc


def build_nc_dm(rep_dt: str, with_bias: bool):
    """Dout-major per-core program: psum partitions = DOUT half, free = TB
    tokens. Halves the matmul instruction count vs token-major (320 matmuls
    of 512 rows) and starts the first chain on 96 KB of data."""
    kc_tot = KC + 1 if with_bias else KC
    dt = F8E3 if rep_dt == "f8e3" else F16
    nc = bacc.Bacc("TRN2", target_bir_lowering=False, debug=False)

    repT_d = nc.dram_tensor("repT", [NB, 128, kc_tot, TB], dt, kind="ExternalInput")
    w2_d = nc.dram_tensor("w2", [128, kc_tot, 2, 128], F16, kind="ExternalInput")
    out_d = nc.dram_tensor("out", [NB, 128, 2, TB], F16, kind="ExternalOutput")

    with tile.TileContext(nc) as tc:
        with (
            tc.tile_pool(name="const", bufs=1) as const_pool,
            tc.tile_pool(name="rep", bufs=4) as rep_pool,
            tc.tile_pool(name="out", bufs=3) as out_pool,
            tc.tile_pool(name="psum", bufs=3, space="PSUM") as psum_pool,
            tc.tile_pool(name="dummy", bufs=1, space="PSUM") as dummy_psum_pool,
        ):
            # Block 0 and w2 split per chunk, interleaved on two HWDGE rings:
            # arrival cadence (~565ns/chunk, sequencer-bound) beats block 0's
            # mid-p-state consumption (~864ns/chunk), so the first chain runs
            # gap-free from the moment chunk 0 (128 KB) lands.
            rep0p, w2p = [None] * kc_tot, [None] * kc_tot
            pieces = [(s, min(2, kc_tot - s)) for s in range(0, kc_tot, 2)]
            for pi, (s, n) in enumerate(pieces):
                r0 = const_pool.tile([128, n, TB], dt, name=f"rep0p{pi}")
                wp = const_pool.tile([128, n, 2, 128], F16, name=f"w2p{pi}")
                nc.sync.dma_start(r0[:], repT_d[0][:, s:s + n, :])
                nc.scalar.dma_start(wp[:], w2_d[:, s:s + n])
                for off in range(n):
                    rep0p[s + off] = (r0, off)
                    w2p[s + off] = (wp, off)

            # PE p-state warm-up (results never read)
            dummy_sb = const_pool.tile([128, DOUT], F16)
            nc.vector.memset(dummy_sb[:], 0.0)
            dummy_ps = dummy_psum_pool.tile([128, DOUT], F32)
            for _ in range(12):
                nc.tensor.matmul(dummy_ps[:], dummy_sb[:, 0:128], dummy_sb[:],
                                 start=True, stop=True)

            def w2_ap(kc, h):
                t, off = w2p[kc]
                return t[:, off, h, :]

            for b in range(NB):
                if b == 0:
                    def rep_ap(kc):
                        t, off = rep0p[kc]
                        return t[:, off, :]
                else:
                    rep_sb = rep_pool.tile([128, kc_tot, TB], dt)
                    nc.sync.dma_start(rep_sb[:], repT_d[b])

                    def rep_ap(kc, _r=rep_sb):
                        return _r[:, kc, :]
                o_sb = out_pool.tile([128, 2, TB], F16)
                ps0 = psum_pool.tile([128, TB], F32, name="ps0")
                ps1 = psum_pool.tile([128, TB], F32, name="ps1")
                # Interleave the two half-chains: bank-alternating matmuls run
                # at 216ns vs 259ns same-bank (the next stationary load hides
                # only across banks). The last block skews h1 four chunks
                # behind h0 (lag-4) so h0's relu + DMA config hide under h1's
                # remaining chain, shortening the drain tail.
                lag = 4 if b == NB - 1 else 0
                seq = []
                for kc in range(kc_tot + lag):
                    if kc < kc_tot:
                        seq.append((0, kc))
                    if kc >= lag:
                        seq.append((1, kc - lag))
                for h, kc in seq:
                    ps = ps0 if h == 0 else ps1
                    nc.tensor.matmul(ps[:], w2_ap(kc, h), rep_ap(kc),
                                     start=kc == 0, stop=kc == kc_tot - 1)
                nc.scalar.activation(o_sb[:, 0, :], ps0[:], AF.Relu)
                if b == NB - 1:
                    nc.scalar.dma_start(out_d[b][:, 0, :], o_sb[:, 0, :])
                    nc.scalar.activation(o_sb[:, 1, :], ps1[:], AF.Relu)
                    nc.scalar.dma_start(out_d[b][:, 1, :], o_sb[:, 1, :])
                else:
                    nc.scalar.activation(o_sb[:, 1, :], ps1[:], AF.Relu)
                    nc.gpsimd.dma_start(out_d[b], o_sb[:])

    nc.compile()
    return nc


def _sigmoid(x):
    out = np.empty_like(x, dtype=np.float32)
    pos = x >= 0
    out[pos] = 1.0 / (1.0 + np.exp(-x[pos]))
    ex = np.exp(x[~pos])
    out[~pos] = ex / (1.0 + ex)
    return out


def prep_all(rep, adj_mask_in, adj_mask_loop, mask, W_in, b_in, W_gate_in,
             b_gate_in, W_self, W_gate_self, adj_arc_in, adj_lab_in):
    """Host prep: gates, gather, K-stack, per-core transpose. Returns
    (in_maps, with_bias)."""
    rep_f = np.ascontiguousarray(np.asarray(rep, np.float32).reshape(BNK * L, DIN))
    adj_arc = np.asarray(adj_arc_in)
    lab = np.asarray(adj_lab_in).reshape(-1)
    idx = (adj_arc[..., 0].reshape(-1) * L + adj_arc[..., 1].reshape(-1)).astype(np.int64)

    b_in = np.asarray(b_in, np.float32)
    with_bias = bool(np.any(b_in != 0.0))

    Wg = np.concatenate([np.asarray(W_gate_in, np.float32),
                         np.asarray(W_gate_self, np.float32)], axis=1)  # [512, 2]
    proj_g = rep_f @ Wg                                                 # [M, 2]
    mk = np.asarray(mask, np.float32).reshape(-1)
    g_in = _sigmoid(proj_g[idx, 0] + np.asarray(b_gate_in, np.float32)[lab, 0])
    g_in *= np.asarray(adj_mask_in, np.float32).reshape(-1) ** 2 * mk
    g_self = _sigmoid(proj_g[:, 1])
    g_self *= np.asarray(adj_mask_loop, np.float32).reshape(-1) ** 2 * mk

    # fold the fp8 pre-scale into the gate vectors (free) and clip in-place
    if REP_DT == "f8e3":
        g_in *= F8_SCALE
        g_self *= F8_SCALE
        w_scale = 1.0 / F8_SCALE
        qdt = ml_dtypes.float8_e3m4
    else:
        w_scale = 1.0
        qdt = np.float16

    def quant(a):
        if REP_DT == "f8e3":
            np.minimum(a, F8_MAX, out=a)
            np.maximum(a, -F8_MAX, out=a)
        return a.astype(qdt)

    rep2 = rep_f[idx]
    rep2 *= g_in[:, None]
    rep3 = rep_f * g_self[:, None]
    blocks = [quant(rep2), quant(rep3)]
    kc_tot = KC
    if with_bias:
        bias_blk = np.zeros((BNK * L, 128), np.float32)
        bias_blk[np.arange(BNK * L), lab] = g_in
        blocks.append(quant(bias_blk))
        kc_tot += 1
    xq = np.concatenate(blocks, axis=1)                                 # [M, kc_tot*128]

    Wstack = [np.asarray(W_in, np.float32), np.asarray(W_self, np.float32)]
    if with_bias:
        Wstack.append(np.concatenate([b_in, np.zeros((128 - NREL, DOUT), np.float32)], axis=0))
    W2 = (np.concatenate(Wstack, axis=0) * w_scale).astype(np.float16)  # [kc_tot*128, 256]

    lay = _eff_layout(with_bias)
    in_maps = []
    if lay == "tm2":
        W2 = np.ascontiguousarray(W2.reshape(kc_tot, 128, DOUT).transpose(1, 0, 2))
        ng = (NTILES - 4) // GROUP
        nphase = kc_tot // 2
        for c in range(NCORES):
            x0 = xq[c * TOK:c * TOK + 4 * TILE_T].reshape(4, TILE_T, nphase, 2, 128)
            rep0x = np.ascontiguousarray(x0.transpose(2, 4, 3, 0, 1))  # [ph, k, kcc, tile, t]
            xc = xq[c * TOK + 4 * TILE_T:(c + 1) * TOK].reshape(ng, GROUP, TILE_T, kc_tot, 128)
            repT = np.ascontiguousarray(xc.transpose(0, 4, 1, 3, 2))
            in_maps.append({"rep0x": rep0x, "repT": repT, "w2": W2})
        return in_maps, with_bias
    if lay == "dm":
        W2 = np.ascontiguousarray(W2.reshape(kc_tot, 128, 2, 128).transpose(1, 0, 2, 3))
        for c in range(NCORES):
            xc = xq[c * TOK:(c + 1) * TOK].reshape(NB, TB, kc_tot, 128)
            repT = np.ascontiguousarray(xc.transpose(0, 3, 2, 1))  # [NB, 128, kc, TB]
            in_maps.append({"repT": repT, "w2": W2})
    else:
        W2 = np.ascontiguousarray(W2.reshape(kc_tot, 128, DOUT).transpose(1, 0, 2))
        for c in range(NCORES):
            xc = xq[c * TOK:(c + 1) * TOK].reshape(NG, GROUP, TILE_T, kc_tot, 128)
            repT = np.ascontiguousarray(xc.transpose(0, 4, 1, 3, 2))  # [NG, 128, G, kc, T]
            in_maps.append({"repT": repT, "w2": W2})
    return in_maps, with_bias


def unshard(results):
    """Per-core device layout -> [BNK, L, DOUT] f32."""
    outs = []
    for r in results:
        if LAYOUT == "dm":
            # [NB, 128(dout half-part), 2(half), TB] -> [TOK, DOUT]
            o = r["out"].astype(np.float32).transpose(0, 3, 2, 1).reshape(TOK, DOUT)
        else:
            # [NG, 128(tok), G, DOUT] -> [TOK, DOUT]
            o = r["out"].astype(np.float32).transpose(0, 2, 1, 3).reshape(TOK, DOUT)
        outs.append(o)
    return np.concatenate(outs, axis=0).reshape(BNK, L, DOUT)


_NC_CACHE = {}


def _eff_layout(with_bias: bool) -> str:
    # tm2's phased start assumes an even chunk count; the 9-chunk bias path
    # falls back to plain tm
    if LAYOUT == "tm2" and with_bias:
        return "tm"
    return LAYOUT


def get_nc(rep_dt: str, with_bias: bool):
    lay = _eff_layout(with_bias)
    key = (lay, rep_dt, with_bias)
    if key not in _NC_CACHE:
        build = {"tm": build_nc, "tm2": build_nc_tm2, "dm": build_nc_dm}[lay]
        _NC_CACHE[key] = build(rep_dt, with_bias)
    return _NC_CACHE[key]


def kernel(rep, adj_mask_in, adj_mask_loop, mask, W_in, b_in, W_gate_in,
           b_gate_in, W_self, W_gate_self, adj_arc_in, adj_lab_in):
    in_maps, with_bias = prep_all(rep, adj_mask_in, adj_mask_loop, mask, W_in,
                                  b_in, W_gate_in, b_gate_in, W_self,
                                  W_gate_self, adj_arc_in, adj_lab_in)
    nc = get_nc(REP_DT, with_bias)
    for _attempt in range(2):
        res = bass_utils.run_bass_kernel_spmd(nc, in_maps, core_ids=list(range(NCORES)))
        out = unshard(res.results)
        if np.isfinite(out).all():
            return out
    return out


# revision 46
# speedup vs baseline: 1.0186x; 1.0186x over previous
"""GCNN message-passing layer on 8 Trainium2 NeuronCores (Bass/Tile).

Math (per token m):
    in_pot[m]  = (rep @ W_in)[head(m)] + b_in[lab(m)]
    in_gate[m] = (rep @ W_gate_in)[head(m)] + b_gate_in[lab(m)]
    self_pot   = rep @ W_self ; self_gate = rep @ W_gate_self
    w_d = sigmoid(gate_d) * msoft_d^2
    out = relu(in_pot*w_in + self_pot*w_self) * mask

Strategy: the gates are 2/514 of the FLOPs, so they're computed on the host
(one [M,512]@[512,2] BLAS call) along with the head gather and the mask
folding. The device input is a single K-stacked operand
    rep23[m] = [ w_in[m]*rep[head(m)] | w_self[m]*rep[m] ]  (K = 1024)
and the kernel reduces to relu(rep23 @ [W_in; W_self]) — one 8-chunk
PSUM-accumulating matmul chain plus one Relu per 128-token tile. No gather
matmuls, no sigmoid/copy tail: the PE array stays at its 2.4 GHz p-state
with nothing else on its critical path.

A nonzero b_in (general path) adds one more K chunk: rep23 gains
w_in[m]*onehot(lab(m)) rows and W2 gains the b_in rows.

Sharding: data-parallel over BNK (160 sentences / core), weights replicated.
The host gather is global, so arbitrary adjacency (even cross-sentence)
is supported.

rep23 ships as fp8-e3m4 (4 mantissa bits) scaled by 3 with the inverse
scale folded into the fp16 weights; W/out stay fp16. This halves the input
DMA (the bottleneck after the matmul restructure). REP_DT='f16' switches
back to all-fp16 if needed.
"""

import os

# Reset cores at device open: long-running sessions can leave the PE clock in
# a degraded state (~1.95 GHz instead of 2.4); a reset restores nominal.
os.environ.setdefault("NEURON_RT_RESET_CORES", "1")

import numpy as np
import ml_dtypes

import concourse.bass as bass  # noqa: F401  (kept for parity with bass_utils expectations)
import concourse.mybir as mybir
import concourse.tile as tile
from concourse import bacc, bass_utils

BNK, L, DIN, DOUT, NREL = 1280, 64, 512, 256, 40
NCORES = 8
SPC = BNK // NCORES          # sentences per core (160)
TOK = SPC * L                # tokens per core (10240)
TILE_T = 128                 # tokens per tile
NTILES = TOK // TILE_T       # 80
GROUP = 2                    # tiles per DMA batch (fine-grained: low head latency)
NG = NTILES // GROUP         # 40
KC = (2 * DIN) // 128        # K chunks for [rep2|rep3] (8)

F32 = mybir.dt.float32
F16 = mybir.dt.float16
F8E3 = mybir.dt.float8e3
AF = mybir.ActivationFunctionType

REP_DT = "f8e3"              # 'f8e3' (half DMA, rel err ~1.3e-2) or 'f16' (~5e-4)
F8_SCALE = 3.0               # rep23 pre-scale; inverse folded into fp16 W2
F8_MAX = 15.5                # e3m4 clamp

LAYOUT = "tm2"                # 'dm' (dout-major, 512-token matmuls) or 'tm' (token-major)
TB = 512                     # tokens per dout-major block (= one PSUM bank in f32)
NB = TOK // TB               # 20 blocks


def build_nc(rep_dt: str, with_bias: bool):
    """Per-core Bass program (identical on all cores)."""
    kc_tot = KC + 1 if with_bias else KC
    dt = F8E3 if rep_dt == "f8e3" else F16
    nc = bacc.Bacc("TRN2", target_bir_lowering=False, debug=False)

    repT_d = nc.dram_tensor("repT", [NG, 128, GROUP, kc_tot, TILE_T], dt, kind="ExternalInput")
    w2_d = nc.dram_tensor("w2", [128, kc_tot, DOUT], F16, kind="ExternalInput")
    out_d = nc.dram_tensor("out", [NG, 128, GROUP, DOUT], F16, kind="ExternalOutput")
    ka = kc_tot // 2  # first-arrival split point for tile 0 / w2

    with tile.TileContext(nc) as tc:
        with (
            tc.tile_pool(name="const", bufs=1) as const_pool,
            tc.tile_pool(name="rep", bufs=8) as rep_pool,
            tc.tile_pool(name="out", bufs=4) as out_pool,
            tc.tile_pool(name="psum", bufs=7, space="PSUM") as psum_pool,
            tc.tile_pool(name="dummy", bufs=1, space="PSUM") as dummy_psum_pool,
        ):
            # Split tile 0's rep and w2 into halves, issued interleaved, so the
            # first matmul chain starts as soon as the first half lands.
            # Dedicated const-pool tiles (not the rotating rep pool) so the
            # first-tile buffers can never alias the steady-state rotation.
            hs = [(0, ka), (ka, kc_tot - ka)]  # (start, len) per half
            rep0 = [const_pool.tile([128, 1, n, TILE_T], dt, name=f"rep0_{h}")
                    for h, (_, n) in enumerate(hs)]
            w2h = [const_pool.tile([128, n, DOUT], F16, name=f"w2_{h}")
                   for h, (_, n) in enumerate(hs)]
            rep0_v = repT_d[0]
            for h, (s, n) in enumerate(hs):
                # rep halves on the SP ring, w2 halves on the ACT ring so the
                # two DGE configs run concurrently at startup
                nc.sync.dma_start(rep0[h][:], rep0_v[:, 0:1, s:s + n, :])
                nc.scalar.dma_start(w2h[h][:], w2_d[:, s:s + n, :])

            # Warm-up matmuls on scratch SBUF (contents irrelevant — results
            # are never read): keep the PE continuously busy through the DMA
            # head so it reaches its full p-state clock before the real
            # chains begin.
            dummy_sb = const_pool.tile([128, DOUT], F16)
            nc.vector.memset(dummy_sb[:], 0.0)
            dummy_ps = dummy_psum_pool.tile([128, DOUT], F32)
            for _ in range(21):
                nc.tensor.matmul(dummy_ps[:], dummy_sb[:, 0:128], dummy_sb[:],
                                 start=True, stop=True)

            def w2_ap(kc):
                h = 0 if kc < ka else 1
                return w2h[h][:, kc - hs[h][0], :]

            for g in range(NG):
                if g == 0:
                    rep_sb = const_pool.tile([128, GROUP - 1, kc_tot, TILE_T], dt, name="rep0b")
                    nc.sync.dma_start(rep_sb[:], rep0_v[:, 1:GROUP])
                    tiles = [lambda kc: rep0[0 if kc < ka else 1][:, 0, kc - hs[0 if kc < ka else 1][0], :]] + [
                        (lambda ti_: lambda kc: rep_sb[:, ti_ - 1, kc, :])(t) for t in range(1, GROUP)]
                else:
                    rep_sb = rep_pool.tile([128, GROUP, kc_tot, TILE_T], dt)
                    nc.sync.dma_start(rep_sb[:], repT_d[g])
                    tiles = [(lambda ti_: lambda kc: rep_sb[:, ti_, kc, :])(t) for t in range(GROUP)]
                o_sb = out_pool.tile([128, GROUP, DOUT], F16)
                for ti in range(GROUP):
                    psum = psum_pool.tile([128, DOUT], F32)
                    for kc in range(kc_tot):
                        nc.tensor.matmul(psum[:], tiles[ti](kc), w2_ap(kc),
                                         start=kc == 0, stop=kc == kc_tot - 1)
                    nc.scalar.activation(o_sb[:, ti, :], psum[:], AF.Relu)
                    # last group drains per-tile on the ACT HWDGE ring for a
                    # shorter tail; earlier groups batch via gpsimd SWDGE below
                    if g == NG - 1:
                        nc.scalar.dma_start(out_d[g][:, ti, :], o_sb[:, ti, :])
                if g != NG - 1:
                    # output DMA via gpsimd SWDGE (idle engine); inputs ride
                    # the SP ring
                    nc.gpsimd.dma_start(out_d[g], o_sb[:])

    nc.compile()
    return nc


def build_nc_tm2(rep_dt: str, with_bias: bool):
    """Token-major with a kc-major phased start: tiles 0-3 ship chunk-pair-
    major and keep four PSUM chains open, so the first matmuls need only
    rep-phase0 (128 KB) + w2 quarter 0 (128 KB) and each 256 KB phase
    sustains 8 matmuls of consumption. Tiles 4+ run the proven tm loop."""
    kc_tot = KC + 1 if with_bias else KC
    assert kc_tot % 2 == 0, "phased start assumes even chunk count"
    dt = F8E3 if rep_dt == "f8e3" else F16
    nphase = kc_tot // 2
    nc = bacc.Bacc("TRN2", target_bir_lowering=False, debug=False)

    ng = (NTILES - 4) // GROUP  # groups for tiles 4+
    rep0x_d = nc.dram_tensor("rep0x", [nphase, 128, 2, 4, TILE_T], dt, kind="ExternalInput")
    repT_d = nc.dram_tensor("repT", [ng, 128, GROUP, kc_tot, TILE_T], dt, kind="ExternalInput")
    w2_d = nc.dram_tensor("w2", [128, kc_tot, DOUT], F16, kind="ExternalInput")
    out_d = nc.dram_tensor("out", [2 + ng, 128, GROUP, DOUT], F16, kind="ExternalOutput")

    with tile.TileContext(nc) as tc:
        with (
            tc.tile_pool(name="const", bufs=1) as const_pool,
            tc.tile_pool(name="rep", bufs=8) as rep_pool,
            tc.tile_pool(name="out", bufs=4) as out_pool,
            tc.tile_pool(name="psum", bufs=7, space="PSUM") as psum_pool,
            tc.tile_pool(name="dummy", bufs=1, space="PSUM") as dummy_psum_pool,
        ):
            # startup stream: rep pieces on the SP ring, w2 pieces on ACT.
            # Chunks 0 and 1 ship as single-chunk pieces (128 KB each with
            # their w2 slice) so the chains start as early as possible; the
            # rest in 2-chunk pieces.
            piece_sizes = [2] * (kc_tot // 2)
            rep0_ap_tbl, w2_ap_tbl = [None] * kc_tot, [None] * kc_tot
            s = 0
            for pi, n in enumerate(piece_sizes):
                rx = const_pool.tile([128, n, 4, TILE_T], dt, name=f"rep0x{pi}")
                wq = const_pool.tile([128, n, DOUT], F16, name=f"w2q{pi}")
                ph, kcc = s // 2, s % 2
                if n == 1:
                    src = rep0x_d[ph][:, kcc:kcc + 1]
                else:
                    assert kcc == 0 and n == 2
                    src = rep0x_d[ph]
                nc.sync.dma_start(rx[:], src)
                nc.scalar.dma_start(wq[:], w2_d[:, s:s + n, :])
                for off in range(n):
                    rep0_ap_tbl[s + off] = (rx, off)
                    w2_ap_tbl[s + off] = (wq, off)
                s += n

            dummy_sb = const_pool.tile([128, DOUT], F16)
            nc.vector.memset(dummy_sb[:], 0.0)
            dummy_ps = dummy_psum_pool.tile([128, DOUT], F32)
            for _ in range(14):
                nc.tensor.matmul(dummy_ps[:], dummy_sb[:, 0:128], dummy_sb[:],
                                 start=True, stop=True)

            # tiles 0-3: four open chains, chunk-major/tile-minor so
            # consecutive matmuls rotate across the four PSUM banks and each
            # arriving chunk unlocks four matmuls
            ps4 = [psum_pool.tile([128, DOUT], F32, name="psum") for _ in range(4)]
            o4 = [out_pool.tile([128, GROUP, DOUT], F16, name=f"o4_{gg}") for gg in range(2)]
            for kc in range(kc_tot):
                rx, roff = rep0_ap_tbl[kc]
                wq, woff = w2_ap_tbl[kc]
                for ti in range(4):
                    nc.tensor.matmul(ps4[ti][:], rx[:, roff, ti, :], wq[:, woff, :],
                                     start=kc == 0, stop=kc == kc_tot - 1)
            for ti in range(4):
                nc.scalar.activation(o4[ti // 2][:, ti % 2, :], ps4[ti][:], AF.Relu)
            for gg in range(2):
                nc.gpsimd.dma_start(out_d[gg], o4[gg][:])

            def w2_ap(kc):
                wq, woff = w2_ap_tbl[kc]
                return wq[:, woff, :]

            for g in range(ng):
                rep_sb = rep_pool.tile([128, GROUP, kc_tot, TILE_T], dt)
                nc.sync.dma_start(rep_sb[:], repT_d[g])
                o_sb = out_pool.tile([128, GROUP, DOUT], F16)
                for ti in range(GROUP):
                    psum = psum_pool.tile([128, DOUT], F32)
                    for kc in range(kc_tot):
                        nc.tensor.matmul(psum[:], rep_sb[:, ti, kc, :], w2_ap(kc),
                                         start=kc == 0, stop=kc == kc_tot - 1)
                    nc.scalar.activation(o_sb[:, ti, :], psum[:], AF.Relu)
                    if g == ng - 1:
                        nc.scalar.dma_start(out_d[2 + g][:, ti, :], o_sb[:, ti, :])
                if g != ng - 1:
                    nc.gpsimd.dma_start(out_d[2 + g], o_sb[:])

    nc.compile()
    return nc


def build_nc_dm(rep_dt: str, with_bias: bool):
    """Dout-major per-core program: psum partitions = DOUT half, free = TB
    tokens. Halves the matmul instruction count vs token-major (320 matmuls
    of 512 rows) and starts the first chain on 96 KB of data."""
    kc_tot = KC + 1 if with_bias else KC
    dt = F8E3 if rep_dt == "f8e3" else F16
    nc = bacc.Bacc("TRN2", target_bir_lowering=False, debug=False)

    repT_d = nc.dram_tensor("repT", [NB, 128, kc_tot, TB], dt, kind="ExternalInput")
    w2_d = nc.dram_tensor("w2", [128, kc_tot, 2, 128], F16, kind="ExternalInput")
    out_d = nc.dram_tensor("out", [NB, 128, 2, TB], F16, kind="ExternalOutput")

    with tile.TileContext(nc) as tc:
        with (
            tc.tile_pool(name="const", bufs=1) as const_pool,
            tc.tile_pool(name="rep", bufs=4) as rep_pool,
            tc.tile_pool(name="out", bufs=3) as out_pool,
            tc.tile_pool(name="psum", bufs=3, space="PSUM") as psum_pool,
            tc.tile_pool(name="dummy", bufs=1, space="PSUM") as dummy_psum_pool,
        ):
            # Block 0 and w2 split per chunk, interleaved on two HWDGE rings:
            # arrival cadence (~565ns/chunk, sequencer-bound) beats block 0's
            # mid-p-state consumption (~864ns/chunk), so the first chain runs
            # gap-free from the moment chunk 0 (128 KB) lands.
            rep0p, w2p = [None] * kc_tot, [None] * kc_tot
            pieces = [(s, min(2, kc_tot - s)) for s in range(0, kc_tot, 2)]
            for pi, (s, n) in enumerate(pieces):
                r0 = const_pool.tile([128, n, TB], dt, name=f"rep0p{pi}")
                wp = const_pool.tile([128, n, 2, 128], F16, name=f"w2p{pi}")
                nc.sync.dma_start(r0[:], repT_d[0][:, s:s + n, :])
                nc.scalar.dma_start(wp[:], w2_d[:, s:s + n])
                for off in range(n):
                    rep0p[s + off] = (r0, off)
                    w2p[s + off] = (wp, off)

            # PE p-state warm-up (results never read)
            dummy_sb = const_pool.tile([128, DOUT], F16)
            nc.vector.memset(dummy_sb[:], 0.0)
            dummy_ps = dummy_psum_pool.tile([128, DOUT], F32)
            for _ in range(12):
                nc.tensor.matmul(dummy_ps[:], dummy_sb[:, 0:128], dummy_sb[:],
                                 start=True, stop=True)

            def w2_ap(kc, h):
                t, off = w2p[kc]
                return t[:, off, h, :]

            for b in range(NB):
                if b == 0:
                    def rep_ap(kc):
                        t, off = rep0p[kc]
                        return t[:, off, :]
                else:
                    rep_sb = rep_pool.tile([128, kc_tot, TB], dt)
                    nc.sync.dma_start(rep_sb[:], repT_d[b])

                    def rep_ap(kc, _r=rep_sb):
                        return _r[:, kc, :]
                o_sb = out_pool.tile([128, 2, TB], F16)
                ps0 = psum_pool.tile([128, TB], F32, name="ps0")
                ps1 = psum_pool.tile([128, TB], F32, name="ps1")
                # Interleave the two half-chains: bank-alternating matmuls run
                # at 216ns vs 259ns same-bank (the next stationary load hides
                # only across banks). The last block skews h1 four chunks
                # behind h0 (lag-4) so h0's relu + DMA config hide under h1's
                # remaining chain, shortening the drain tail.
                lag = 4 if b == NB - 1 else 0
                seq = []
                for kc in range(kc_tot + lag):
                    if kc < kc_tot:
                        seq.append((0, kc))
                    if kc >= lag:
                        seq.append((1, kc - lag))
                for h, kc in seq:
                    ps = ps0 if h == 0 else ps1
                    nc.tensor.matmul(ps[:], w2_ap(kc, h), rep_ap(kc),
                                     start=kc == 0, stop=kc == kc_tot - 1)
                nc.scalar.activation(o_sb[:, 0, :], ps0[:], AF.Relu)
                if b == NB - 1:
                    nc.scalar.dma_start(out_d[b][:, 0, :], o_sb[:, 0, :])
                    nc.scalar.activation(o_sb[:, 1, :], ps1[:], AF.Relu)
                    nc.scalar.dma_start(out_d[b][:, 1, :], o_sb[:, 1, :])
                else:
                    nc.scalar.activation(o_sb[:, 1, :], ps1[:], AF.Relu)
                    nc.gpsimd.dma_start(out_d[b], o_sb[:])

    nc.compile()
    return nc


def _sigmoid(x):
    out = np.empty_like(x, dtype=np.float32)
    pos = x >= 0
    out[pos] = 1.0 / (1.0 + np.exp(-x[pos]))
    ex = np.exp(x[~pos])
    out[~pos] = ex / (1.0 + ex)
    return out


def prep_all(rep, adj_mask_in, adj_mask_loop, mask, W_in, b_in, W_gate_in,
             b_gate_in, W_self, W_gate_self, adj_arc_in, adj_lab_in):
    """Host prep: gates, gather, K-stack, per-core transpose. Returns
    (in_maps, with_bias)."""
    rep_f = np.ascontiguousarray(np.asarray(rep, np.float32).reshape(BNK * L, DIN))
    adj_arc = np.asarray(adj_arc_in)
    lab = np.asarray(adj_lab_in).reshape(-1)
    idx = (adj_arc[..., 0].reshape(-1) * L + adj_arc[..., 1].reshape(-1)).astype(np.int64)

    b_in = np.asarray(b_in, np.float32)
    with_bias = bool(np.any(b_in != 0.0))

    Wg = np.concatenate([np.asarray(W_gate_in, np.float32),
                         np.asarray(W_gate_self, np.float32)], axis=1)  # [512, 2]
    proj_g = rep_f @ Wg                                                 # [M, 2]
    mk = np.asarray(mask, np.float32).reshape(-1)
    g_in = _sigmoid(proj_g[idx, 0] + np.asarray(b_gate_in, np.float32)[lab, 0])
    g_in *= np.asarray(adj_mask_in, np.float32).reshape(-1) ** 2 * mk
    g_self = _sigmoid(proj_g[:, 1])
    g_self *= np.asarray(adj_mask_loop, np.float32).reshape(-1) ** 2 * mk

    # fold the fp8 pre-scale into the gate vectors (free) and clip in-place
    if REP_DT == "f8e3":
        g_in *= F8_SCALE
        g_self *= F8_SCALE
        w_scale = 1.0 / F8_SCALE
        qdt = ml_dtypes.float8_e3m4
    else:
        w_scale = 1.0
        qdt = np.float16

    def quant(a):
        if REP_DT == "f8e3":
            np.minimum(a, F8_MAX, out=a)
            np.maximum(a, -F8_MAX, out=a)
        return a.astype(qdt)

    rep2 = rep_f[idx]
    rep2 *= g_in[:, None]
    rep3 = rep_f * g_self[:, None]
    blocks = [quant(rep2), quant(rep3)]
    kc_tot = KC
    if with_bias:
        bias_blk = np.zeros((BNK * L, 128), np.float32)
        bias_blk[np.arange(BNK * L), lab] = g_in
        blocks.append(quant(bias_blk))
        kc_tot += 1
    xq = np.concatenate(blocks, axis=1)                                 # [M, kc_tot*128]

    Wstack = [np.asarray(W_in, np.float32), np.asarray(W_self, np.float32)]
    if with_bias:
        Wstack.append(np.concatenate([b_in, np.zeros((128 - NREL, DOUT), np.float32)], axis=0))
    W2 = (np.concatenate(Wstack, axis=0) * w_scale).astype(np.float16)  # [kc_tot*128, 256]

    lay = _eff_layout(with_bias)
    in_maps = []
    if lay == "tm2":
        W2 = np.ascontiguousarray(W2.reshape(kc_tot, 128, DOUT).transpose(1, 0, 2))
        ng = (NTILES - 4) // GROUP
        nphase = kc_tot // 2
        for c in range(NCORES):
            x0 = xq[c * TOK:c * TOK + 4 * TILE_T].reshape(4, TILE_T, nphase, 2, 128)
            rep0x = np.ascontiguousarray(x0.transpose(2, 4, 3, 0, 1))  # [ph, k, kcc, tile, t]
            xc = xq[c * TOK + 4 * TILE_T:(c + 1) * TOK].reshape(ng, GROUP, TILE_T, kc_tot, 128)
            repT = np.ascontiguousarray(xc.transpose(0, 4, 1, 3, 2))
            in_maps.append({"rep0x": rep0x, "repT": repT, "w2": W2})
        return in_maps, with_bias
    if lay == "dm":
        W2 = np.ascontiguousarray(W2.reshape(kc_tot, 128, 2, 128).transpose(1, 0, 2, 3))
        for c in range(NCORES):
            xc = xq[c * TOK:(c + 1) * TOK].reshape(NB, TB, kc_tot, 128)
            repT = np.ascontiguousarray(xc.transpose(0, 3, 2, 1))  # [NB, 128, kc, TB]
            in_maps.append({"repT": repT, "w2": W2})
    else:
        W2 = np.ascontiguousarray(W2.reshape(kc_tot, 128, DOUT).transpose(1, 0, 2))
        for c in range(NCORES):
            xc = xq[c * TOK:(c + 1) * TOK].reshape(NG, GROUP, TILE_T, kc_tot, 128)
            repT = np.ascontiguousarray(xc.transpose(0, 4, 1, 3, 2))  # [NG, 128, G, kc, T]
            in_maps.append({"repT": repT, "w2": W2})
    return in_maps, with_bias


def unshard(results):
    """Per-core device layout -> [BNK, L, DOUT] f32."""
    outs = []
    for r in results:
        if LAYOUT == "dm":
            # [NB, 128(dout half-part), 2(half), TB] -> [TOK, DOUT]
            o = r["out"].astype(np.float32).transpose(0, 3, 2, 1).reshape(TOK, DOUT)
        else:
            # [NG, 128(tok), G, DOUT] -> [TOK, DOUT]
            o = r["out"].astype(np.float32).transpose(0, 2, 1, 3).reshape(TOK, DOUT)
        outs.append(o)
    return np.concatenate(outs, axis=0).reshape(BNK, L, DOUT)


_NC_CACHE = {}


def _eff_layout(with_bias: bool) -> str:
    # tm2's phased start assumes an even chunk count; the 9-chunk bias path
    # falls back to plain tm
    if LAYOUT == "tm2" and with_bias:
        return "tm"
    return LAYOUT


def get_nc(rep_dt: str, with_bias: bool):
    lay = _eff_layout(with_bias)
    key = (lay, rep_dt, with_bias)
    if key not in _NC_CACHE:
        build = {"tm": build_nc, "tm2": build_nc_tm2, "dm": build_nc_dm}[lay]
        _NC_CACHE[key] = build(rep_dt, with_bias)
    return _NC_CACHE[key]


def kernel(rep, adj_mask_in, adj_mask_loop, mask, W_in, b_in, W_gate_in,
           b_gate_in, W_self, W_gate_self, adj_arc_in, adj_lab_in):
    in_maps, with_bias = prep_all(rep, adj_mask_in, adj_mask_loop, mask, W_in,
                                  b_in, W_gate_in, b_gate_in, W_self,
                                  W_gate_self, adj_arc_in, adj_lab_in)
    nc = get_nc(REP_DT, with_bias)
    for _attempt in range(2):
        res = bass_utils.run_bass_kernel_spmd(nc, in_maps, core_ids=list(range(NCORES)))
        out = unshard(res.results)
        if np.isfinite(out).all():
            return out
    return out
